# revision 32
# baseline (speedup 1.0000x reference)
"""Causal self-attention (B=4, T=2048, C=1024, 16 heads) on 8 TRN2 NeuronCores.

Sharding: tensor-parallel over heads. Each core owns 2 heads (128 of the
1024 q/k/v dims): wq/wk/wv are split by rows (output dim), wo by columns.
Each core computes a full [C, B*T] partial of the output projection; the
host sums the 8 partials.

On-core layout is "transposed": activations live as [feature, token] so
every matmul has tokens on the moving free dim (>=256 wide -> float32r
matmuls run at 1 cycle/row). Attention is computed as s^T = K Q^T with
keys on partitions; softmax max-subtraction is skipped (logits are O(10),
exp is safe in fp32) and the denominator comes from a ones-column
appended to V in the P^T @ V matmul. Causal masking replaces masked
probabilities with exp(-10) (the module masks logits with -10, not -inf).
Probabilities and V run in bf16 (denominator and numerator use the same
quantized probs, so the bias largely cancels); projections and scores
stay in f32r.
"""

import os
import sys

import numpy as np

for _p in ("/opt/trn_rl_repo",):
    if _p not in sys.path and os.path.isdir(_p):
        sys.path.insert(0, _p)

_B, _T, _C = 4, 2048, 1024
_NHEAD, _HD = 16, 64
_NC = 8
_LOC = (_NHEAD // _NC) * _HD  # feature dims per core = 128 (2 heads)
_BT = _B * _T                 # 8192 tokens
_TC = 512                     # token chunk (psum bank / moving-operand width)
_NTC = _BT // _TC             # 16 projection chunks
_KC = _C // 128               # 8 contraction chunks over the embedding
_NQC = _T // _TC              # 4 query chunks per batch
_NKB = _T // 128              # 16 key blocks per batch
_EXPM = float(np.exp(-10.0))  # exp of the mask fill value

TRACE = bool(int(os.environ.get("KERNEL_TRACE", "0")))
LAST_EXEC_NS = None
LAST_RESULTS = None

_cache = {}


def _build():
    import concourse.mybir as mybir
    import concourse.tile as tile
    from concourse import bacc

    f32 = mybir.dt.float32
    f32r = mybir.dt.float32r
    bf16 = mybir.dt.bfloat16
    AF = mybir.ActivationFunctionType

    nc = bacc.Bacc("TRN2", target_bir_lowering=False, debug=False)

    xT_d = nc.dram_tensor("xT", [_C, _BT], f32r, kind="ExternalInput").ap()
    wqT_d = nc.dram_tensor("wqT", [_C, _LOC], f32r, kind="ExternalInput").ap()
    wkT_d = nc.dram_tensor("wkT", [_C, _LOC], f32r, kind="ExternalInput").ap()
    wvT_d = nc.dram_tensor("wvT", [_C, _LOC], f32r, kind="ExternalInput").ap()
    woT_d = nc.dram_tensor("woT", [_LOC, _C], f32r, kind="ExternalInput").ap()
    idc_d = nc.dram_tensor("identc", [128, 64], f32r, kind="ExternalInput").ap()
    oneb_d = nc.dram_tensor("onesb", [128, 1], bf16, kind="ExternalInput").ap()
    oner_d = nc.dram_tensor("onesr", [1, 64], f32r, kind="ExternalInput").ap()
    outT_d = nc.dram_tensor("outT", [_C, _BT], f32, kind="ExternalOutput").ap()

    xT_v = xT_d.rearrange("(c p) n -> p c n", p=128)    # [128, 8, 8192]
    wq_v = wqT_d.rearrange("(c p) m -> p c m", p=128)   # [128, 8, 128]
    wk_v = wkT_d.rearrange("(c p) m -> p c m", p=128)
    wv_v = wvT_d.rearrange("(c p) m -> p c m", p=128)
    wo_v = woT_d.rearrange("p (m n) -> p m n", n=128)   # [128, 8, 128]

    with tile.TileContext(nc) as tc:
        with (
            tc.tile_pool(name="consts", bufs=1) as cp,
            tc.tile_pool(name="sb", bufs=2) as sp,
            tc.tile_pool(name="ps", bufs=2, space="PSUM") as pp,
        ):
            # first token chunk of x starts loading before anything else
            xa0 = sp.tile([128, 4, _TC], f32r, tag="xa", bufs=2)
            xb0 = sp.tile([128, 4, _TC], f32r, tag="xb", bufs=2)
            for c4 in range(4):
                nc.sync.dma_start(xa0[:, c4, :], xT_v[:, c4, 0:_TC])
                nc.sync.dma_start(xb0[:, c4, :], xT_v[:, 4 + c4, 0:_TC])
            w_sb = {}
            for nm, v in (("q", wq_v), ("k", wk_v), ("v", wv_v)):
                t = cp.tile([128, _KC, 128], f32r, tag=f"w{nm}")
                nc.sync.dma_start(t[:], v[:])
                w_sb[nm] = t
            wo_sb = cp.tile([128, _KC, 128], f32r, tag="wo")
            nc.sync.dma_start(wo_sb[:], wo_v[:])
            ident = cp.tile([128, 64], f32r, tag="ident")
            nc.sync.dma_start(ident[:], idc_d[:])
            ones1 = cp.tile([65, 64], f32r, tag="ones1")
            nc.sync.dma_start(ones1[64:65, :], oner_d[:])
            ones_b = cp.tile([128, 1], bf16, tag="onesb")
            nc.sync.dma_start(ones_b[:], oneb_d[:])

            qT = cp.tile([128, _BT], f32r, tag="qT")
            kT = cp.tile([128, _BT], f32r, tag="kT")
            # v in [token, dim] layout per 128-token block, per head, with a
            # trailing ones column (row sums -> softmax denominator)
            vaug = cp.tile([128, _BT // 128, 2, 65], bf16, tag="vaug")
            for h in range(2):
                nc.sync.dma_start(
                    vaug[:, :, h, 64:65],
                    oneb_d[:, 0:1].to_broadcast([128, _BT // 128, 1]),
                )

            # ---------------- q/k/v projections ----------------
            for t in range(_NTC):
                tok = slice(t * _TC, (t + 1) * _TC)
                if t == 0:
                    halves = (xa0, xb0)
                else:
                    xa = sp.tile([128, 4, _TC], f32r, tag="xa", bufs=2)
                    xb = sp.tile([128, 4, _TC], f32r, tag="xb", bufs=2)
                    for c4 in range(4):
                        nc.sync.dma_start(xa[:, c4, :], xT_v[:, c4, tok])
                        nc.sync.dma_start(xb[:, c4, :], xT_v[:, 4 + c4, tok])
                    halves = (xa, xb)
                for nm in ("q", "k", "v"):
                    ps = pp.tile([128, _TC], f32, tag="psC", bufs=2)
                    for c in range(_KC):
                        nc.tensor.matmul(
                            ps[:],
                            w_sb[nm][:, c, :],
                            halves[c // 4][:, c % 4, :],
                            start=(c == 0),
                            stop=(c == _KC - 1),
                        )
                    if nm == "q":
                        nc.vector.tensor_copy(qT[:, tok], ps[:])
                    elif nm == "k":
                        nc.vector.tensor_copy(kT[:, tok], ps[:])
                    else:
                        vtc = sp.tile([128, _TC], f32r, tag="vtc", bufs=2)
                        nc.vector.tensor_copy(vtc[:], ps[:])
                        for h in range(2):
                            tp = pp.tile([128, 4, 64], f32, tag="psC",
                                         bufs=2)
                            for s4 in range(4):
                                nc.tensor.transpose(
                                    tp[:, s4, :].bitcast(f32r),
                                    vtc[h * 64:(h + 1) * 64,
                                        s4 * 128:(s4 + 1) * 128],
                                    ident[h * 64:(h + 1) * 64, :],
                                )
                            nc.vector.tensor_copy(
                                vaug[:, t * 4:t * 4 + 4, h, 0:64], tp[:]
                            )

            # ---------------- attention + output projection ----------------
            for b in range(_B):
                ycat = sp.tile([128, _T], f32r, tag="ycat", bufs=2)
                for h in range(2):
                    rows = slice(h * 64, (h + 1) * 64)
                    ytmp = sp.tile([65, _T], f32r, tag="ytmp", bufs=2)
                    # column sums of v over each chunk's fully-masked key
                    # blocks, accumulated in PSUM: suf[:, c] = sum over
                    # kb >= 4c+4 of (v_kb^T @ 1).  Applied (scaled by
                    # exp(-10)) as a bias when copying y out of PSUM.
                    suf_ps = pp.tile([65, _NQC - 1], f32, tag="suf", bufs=1)
                    for c in range(_NQC - 1):
                        for kb in range(4 * c + 4, _NKB):
                            nc.tensor.matmul(
                                suf_ps[:, c:c + 1],
                                vaug[:, b * 16 + kb, h, :],
                                ones_b[:],
                                start=(kb == 4 * c + 4),
                                stop=(kb == _NKB - 1),
                            )
                    suf_sb = sp.tile([65, _NQC - 1], f32, tag="suf", bufs=2)
                    nc.scalar.activation(
                        suf_sb[:], suf_ps[:], AF.Copy, scale=_EXPM
                    )
                    for c in range(_NQC):
                        qc = slice(b * _T + c * _TC, b * _T + (c + 1) * _TC)
                        cc = slice(c * _TC, (c + 1) * _TC)
                        yps = pp.tile([65, _TC], f32, tag="yT", bufs=1)
                        for kb in range(4 * c + 4):
                            j = kb - 4 * c
                            sps = pp.tile([128, _TC], f32, tag="psB",
                                          bufs=4)
                            # band blocks j=1,2: the leading 128j columns
                            # are fully masked, so the score matmul only
                            # needs the tail (tail >= 256 keeps f32r fast)
                            off = 128 * j if j in (1, 2) else 0
                            nc.tensor.matmul(
                                sps[:, off:],
                                kT[rows,
                                   b * _T + kb * 128:
                                   b * _T + (kb + 1) * 128],
                                qT[rows,
                                   b * _T + c * _TC + off:
                                   b * _T + (c + 1) * _TC],
                                start=True, stop=True,
                            )
                            pexp = sp.tile([128, _TC], bf16, tag="pexp",
                                           bufs=16)
                            if j >= 1:
                                # leading 128j columns are fully masked;
                                # the affine_select fills them below
                                nc.scalar.activation(
                                    pexp[:, 128 * j:], sps[:, 128 * j:],
                                    AF.Exp, scale=0.125
                                )
                            else:
                                nc.scalar.activation(
                                    pexp[:], sps[:], AF.Exp, scale=0.125
                                )
                            if j >= 0:
                                # causal: keep where qi - ki - 128j >= 0,
                                # else fill exp(-10); columns right of the
                                # diagonal strip are always valid
                                w = 128 * (j + 1)
                                nc.gpsimd.affine_select(
                                    out=pexp[:, 0:w],
                                    in_=pexp[:, 0:w],
                                    compare_op=mybir.AluOpType.is_ge,
                                    fill=_EXPM,
                                    base=-128 * j,
                                    pattern=[[1, w]],
                                    channel_multiplier=-1,
                                )
                            nc.tensor.matmul(
                                yps[:],
                                vaug[:, b * 16 + kb, h, :],
                                pexp[:],
                                start=(kb == 0),
                                stop=(kb == 4 * c + 3),
                            )
                        if c < _NQC - 1:
                            nc.scalar.activation(
                                ytmp[:, cc], yps[:], AF.Identity,
                                bias=suf_sb[:, c:c + 1],
                            )
                        else:
                            nc.scalar.copy(ytmp[:, cc], yps[:])
                    # normalize: row 64 holds the softmax denominator;
                    # broadcast Z over the 64 dims via a K=1 matmul, then
                    # reciprocal + multiply per chunk
                    for c in range(_NQC):
                        cc = slice(c * _TC, (c + 1) * _TC)
                        zps = pp.tile([64, _TC], f32, tag="psC", bufs=2)
                        nc.tensor.matmul(
                            zps[:],
                            ones1[64:65, :],
                            ytmp[64:65, cc],
                            start=True, stop=True,
                        )
                        zrec = sp.tile([64, _TC], f32, tag="zrec", bufs=2)
                        nc.vector.reciprocal(zrec[:], zps[:])
                        nc.vector.tensor_mul(
                            ycat[rows, cc], ytmp[0:64, cc], zrec[:]
                        )
                # last batch: chunk-outer order starts the output drain
                # as soon as each ycat chunk is normalized
                if b == _B - 1:
                    mc2 = [(m, c2) for c2 in range(_NQC)
                           for m in range(_KC)]
                else:
                    mc2 = [(m, c2) for m in range(_KC)
                           for c2 in range(_NQC)]
                for m, c2 in mc2:
                    ops = pp.tile([128, _TC], f32, tag="psC", bufs=2)
                    nc.tensor.matmul(
                        ops[:],
                        wo_sb[:, m, :],
                        ycat[:, c2 * _TC:(c2 + 1) * _TC],
                        start=True, stop=True,
                    )
                    ostg = sp.tile([128, _TC], f32, tag="ostg", bufs=6)
                    if b == _B - 1 and c2 % 2 == 0:
                        nc.scalar.copy(ostg[:], ops[:])
                    else:
                        nc.vector.tensor_copy(ostg[:], ops[:])
                    nc.sync.dma_start(
                        outT_d[m * 128:(m + 1) * 128,
                               b * _T + c2 * _TC:b * _T + (c2 + 1) * _TC],
                        ostg[:],
                    )

    nc.compile()
    return nc, outT_d.name


def _get_nc():
    if "nc" not in _cache:
        _cache["nc"] = _build()
    return _cache["nc"]


def kernel(**inputs):
    import ml_dtypes

    from concourse.bass_utils import run_bass_kernel_spmd

    x = np.ascontiguousarray(np.asarray(inputs["x"]), dtype=np.float32)
    wq = np.ascontiguousarray(np.asarray(inputs["wq"]), dtype=np.float32)
    wk = np.ascontiguousarray(np.asarray(inputs["wk"]), dtype=np.float32)
    wv = np.ascontiguousarray(np.asarray(inputs["wv"]), dtype=np.float32)
    wo = np.ascontiguousarray(np.asarray(inputs["wo"]), dtype=np.float32)

    xT = np.ascontiguousarray(x.reshape(_BT, _C).T)
    identc = np.zeros((128, 64), dtype=np.float32)
    identc[np.arange(128), np.arange(128) % 64] = 1.0
    onesb = np.ones((128, 1), dtype=ml_dtypes.bfloat16)
    onesr = np.ones((1, 64), dtype=np.float32)

    in_maps = []
    for i in range(_NC):
        r = slice(_LOC * i, _LOC * (i + 1))
        in_maps.append({
            "xT": xT,
            "wqT": np.ascontiguousarray(wq[r].T),
            "wkT": np.ascontiguousarray(wk[r].T),
            "wvT": np.ascontiguousarray(wv[r].T),
            "woT": np.ascontiguousarray(wo[:, r].T),
            "identc": identc,
            "onesb": onesb,
            "onesr": onesr,
        })

    nc, outname = _get_nc()
    try:
        res = run_bass_kernel_spmd(nc, in_maps, list(range(_NC)), trace=TRACE)
    except ModuleNotFoundError:
        # NTFF profiling hook unavailable in this container
        res = run_bass_kernel_spmd(nc, in_maps, list(range(_NC)), trace=False)

    global LAST_EXEC_NS, LAST_RESULTS
    LAST_EXEC_NS = res.exec_time_ns
    LAST_RESULTS = res

    acc = np.zeros((_C, _BT), dtype=np.float64)
    for i in range(_NC):
        acc += res.results[i][outname]
    return np.ascontiguousarray(acc.T).reshape(_B, _T, _C).astype(np.float32)

